# revision 31
# baseline (speedup 1.0000x reference)
"""Block-sparse linear kernel for Trainium2 (8 NeuronCores, Bass/Tile).

Computes out = x @ (weight*mask).T + bias for
  x [4, 2048, 4096] f32, weight [4096, 4096] f32, mask [4096,4096] bool,
  bias [4096] f32  ->  out [4, 2048, 4096] f32.

Strategy (data-parallel over tokens, 8 cores x 1024 tokens each):
  The 16x16 block mask is coarsened by greedy max-overlap matching into
  supercells of 2 input blocks (K=32) x 4 output blocks (M=64). Only
  nonzero supercells are computed, as [32,64,512] PE-tiled fp16 matmuls
  (fp32 accumulate in PSUM) on 8 concurrent tensor-engine slots
  (4 row groups x 2 column positions). ~3575 supercells vs 16384 dense
  equivalents => ~1.9x over a dense roofline kernel.

  Per core: x resident in SBUF as [128, 32, 1024] fp16 (input pair i at
  partition group i%4, ko i//4). 64 output groups processed in 32 sets of
  2; slot (r, cpos) accumulates into psum bank ps[chunk][r] partitions
  [64*cpos:64*cpos+64). Token chunks (2x512) are processed sequentially so
  chunk-0 psum drains overlap chunk-1 compute. Drain = 1 ScalarE
  activation (bias add) + 3 VectorE adds, then DMA out.

  Falls back to a dense fp16 kernel when the mask is not sparse enough.
"""

import sys

for _p in ("/opt/trn_rl_repo",):
    if _p not in sys.path:
        sys.path.insert(0, _p)

import numpy as np

import concourse.bacc as bacc
import concourse.mybir as mybir
import concourse.tile as tile
from concourse import bass_utils

P = 128
IN = 4096
OUT = 4096
BLK = 16
NB = IN // BLK  # 256 blocks per dim
NPAIR = NB // 2  # 128 input pairs
KO = IN // P  # 32
M = 64  # out-features per supercell
NG = OUT // M  # 64 output groups
NSET = NG // 2  # 32 sets (2 col positions)
N_CORES = 8
TOK = 1024
NCHUNK = 512
NT = TOK // NCHUNK  # 2
F16 = mybir.dt.float16
F32 = mybir.dt.float32

# sparse path wins while 2 * supercells * ~40ns < dense ~450us
SPARSE_MAX_CELLS = 5400


# ---------------------------------------------------------------- matching


def greedy_pair(support):
    """support: [N, D] bool rows. Pair rows maximizing overlap; [N/2, 2]."""
    N = support.shape[0]
    A = support.astype(np.int32)
    O = A @ A.T
    np.fill_diagonal(O, -1)
    pairs = []
    for _ in range(N // 2):
        idx = int(np.argmax(O))
        i, j = divmod(idx, N)
        pairs.append((i, j))
        O[i, :] = -1
        O[:, i] = -1
        O[j, :] = -1
        O[:, j] = -1
    return np.array(pairs, dtype=np.int64)


def analyze_mask(mask):
    """Returns (in_pairs [128,2], groups [64][4 block ids], sc64 [64,128] bool).

    Alternates re-grouping outputs against current input pairs and
    re-pairing inputs against current output groups, keeping the best.
    """
    bm = mask.reshape(NB, BLK, NB, BLK).any(axis=(1, 3))  # [out_blk, in_blk]
    in_pairs = greedy_pair(bm.T)
    best = None
    for _ in range(4):
        # group outputs (4 blocks each) against current input pairs
        bmc = bm[:, in_pairs[:, 0]] | bm[:, in_pairs[:, 1]]  # [256, 128]
        out_pairs = greedy_pair(bmc)
        sc32 = bmc[out_pairs[:, 0]] | bmc[out_pairs[:, 1]]
        rp = greedy_pair(sc32)
        sc64 = sc32[rp[:, 0]] | sc32[rp[:, 1]]  # [64, 128]
        groups = np.array(
            [[out_pairs[a][0], out_pairs[a][1], out_pairs[b][0], out_pairs[b][1]]
             for a, b in rp], dtype=np.int64)
        cells = int(sc64.sum())
        if best is None or cells < best[0]:
            best = (cells, in_pairs.copy(), groups, sc64)
        # re-pair inputs against the output groups
        bg = np.zeros((NG, NB), dtype=bool)  # [group, in_blk]
        for g in range(NG):
            bg[g] = bm[groups[g]].any(axis=0)
        in_pairs = greedy_pair(bg.T)
    _, in_pairs, groups, _ = best

    # refine by alternating hill-climbs: output-block<->group swaps and
    # input-block<->pair swaps, both scored on total nonzero cells
    groups = groups.copy()
    in_pairs = np.array(in_pairs)
    rng = np.random.default_rng(1)

    def refine_groups(ip, groups, iters):
        bp = bm[:, ip[:, 0]] | bm[:, ip[:, 1]]  # [out_blk, pair]
        cnt = np.zeros((NG, NPAIR), dtype=np.int16)
        for g in range(NG):
            cnt[g] = bp[groups[g]].sum(axis=0)
        gi = np.zeros(NB, dtype=np.int64)
        pos = np.zeros(NB, dtype=np.int64)
        for g in range(NG):
            for k in range(4):
                gi[groups[g][k]] = g
                pos[groups[g][k]] = k
        for _ in range(iters):
            u, v = rng.integers(0, NB, 2)
            g1, g2 = gi[u], gi[v]
            if g1 == g2:
                continue
            n1 = cnt[g1] - bp[u] + bp[v]
            n2 = cnt[g2] - bp[v] + bp[u]
            d = (int((n1 > 0).sum()) + int((n2 > 0).sum())
                 - int((cnt[g1] > 0).sum()) - int((cnt[g2] > 0).sum()))
            if d <= 0:
                cnt[g1], cnt[g2] = n1, n2
                k1, k2 = pos[u], pos[v]
                groups[g1][k1], groups[g2][k2] = v, u
                gi[u], gi[v] = g2, g1
                pos[u], pos[v] = k2, k1
        return groups

    def refine_ip(ip, groups, iters):
        bg = np.zeros((NG, NB), dtype=bool)
        for g in range(NG):
            bg[g] = bm[groups[g]].any(axis=0)
        S = bg.T  # [in_blk, group]
        pairs = [tuple(p) for p in ip]

        def pcost(a, b):
            return int((S[a] | S[b]).sum())

        cost = np.array([pcost(a, b) for a, b in pairs])
        pos = np.empty(NB, dtype=np.int64)
        for k, (a, b) in enumerate(pairs):
            pos[a] = pos[b] = k
        for _ in range(iters):
            u, v = rng.integers(0, NB, 2)
            if u == v or pos[u] == pos[v]:
                continue
            ku, kv = pos[u], pos[v]
            au, bu = pairs[ku]
            av, bv = pairs[kv]
            pu = bu if au == u else au
            pv = bv if av == v else av
            c1, c2 = pcost(v, pu), pcost(u, pv)
            d = c1 + c2 - cost[ku] - cost[kv]
            if d <= 0:
                pairs[ku] = (v, pu)
                pairs[kv] = (u, pv)
                cost[ku], cost[kv] = c1, c2
                pos[u], pos[v] = kv, ku
        return np.array(pairs, dtype=np.int64)

    def cells_of(ip, groups):
        bp = bm[:, ip[:, 0]] | bm[:, ip[:, 1]]
        g = np.zeros((NG, NPAIR), dtype=bool)
        for G in range(NG):
            g[G] = bp[groups[G]].any(axis=0)
        return int(g.sum()), g

    groups = refine_groups(in_pairs, groups, 60000)
    best = (*cells_of(in_pairs, groups), in_pairs.copy(), groups.copy())
    for _ in range(5):
        in_pairs = refine_ip(in_pairs, groups, 60000)
        groups = refine_groups(in_pairs, groups, 60000)
        c, g = cells_of(in_pairs, groups)
        if c < best[0]:
            best = (c, g, in_pairs.copy(), groups.copy())
    _, sc64, in_pairs, groups = best
    return in_pairs, groups, sc64


def _rebalance_pairs(sc64, set_G):
    """Permute pair layout positions to even out per-set slot loads."""
    rng = np.random.default_rng(0)
    perm = np.arange(NPAIR)
    use_ids = [[np.nonzero(sc64[set_G[s][c]])[0] for c in range(2)]
               for s in range(NSET)]

    def set_cost(p):
        rg = np.zeros(NPAIR, dtype=np.int64)
        rg[p] = np.arange(NPAIR) % 4
        tot = 0
        for s in range(NSET):
            mx = 1
            for c in range(2):
                ids = use_ids[s][c]
                if len(ids):
                    mx = max(mx, int(np.bincount(rg[ids], minlength=4).max()))
            tot += mx
        return tot

    best = set_cost(perm)
    for _ in range(4000):
        i, j = rng.integers(0, NPAIR, 2)
        if i == j or (i % 4) == (j % 4):
            continue
        perm[[i, j]] = perm[[j, i]]
        c = set_cost(perm)
        if c <= best:
            best = c
        else:
            perm[[i, j]] = perm[[j, i]]
    return perm


def build_schedule(sc64):
    counts = sc64.sum(axis=1)
    order = np.argsort(-counts)
    set_G = [[int(order[s * 2 + c]) for c in range(2)] for s in range(NSET)]
    perm = _rebalance_pairs(sc64, set_G)
    pair_pos = np.empty(NPAIR, dtype=np.int64)
    pair_pos[perm] = np.arange(NPAIR)
    slot_lists = []
    for s in range(NSET):
        rows = []
        for r in range(4):
            cols = []
            for c in range(2):
                G = set_G[s][c]
                cs = sorted(int(pair_pos[i]) for i in np.nonzero(sc64[G])[0]
                            if pair_pos[i] % 4 == r)
                cols.append(cs)
            rows.append(cols)
        slot_lists.append(rows)
    return set_G, slot_lists, perm


# ---------------------------------------------------------------- device


def build_merge_plan(slot_lists):
    """Merge vertically-adjacent cells (same ko, row groups 2R/2R+1, same
    column) into K=64 matmuls. Entry = (kind 'm'/'s', c, pos); merged
    entries of rows 2R and 2R+1 are offset-aligned."""
    plans = []
    for s in range(NSET):
        rows = [[] for _ in range(4)]
        for R in range(2):
            r0, r1 = 2 * R, 2 * R + 1
            merged = []
            for c in range(2):
                l0 = set(slot_lists[s][r0][c])
                l1 = set(slot_lists[s][r1][c])
                for q in sorted(l0):
                    if q + 1 in l1:
                        merged.append((c, q))
            rows[r0] = [("m", c, q) for c, q in merged]
            rows[r1] = [("m", c, q + 1) for c, q in merged]
            for c in range(2):
                ml0 = {q for cc, q in merged if cc == c}
                rows[r0] += [("s", c, q) for q in sorted(slot_lists[s][r0][c])
                             if q not in ml0]
                rows[r1] += [("s", c, q) for q in sorted(slot_lists[s][r1][c])
                             if q - 1 not in ml0]
        for r in range(4):
            for c in range(2):
                writers = sum(1 for k, cc, q in rows[r]
                              if cc == c and (k == "s" or r % 2 == 0))
                if writers == 0:
                    rows[r].append(("s", c, -1))
        plans.append(rows)
    return plans


def schedule_items(rows):
    """Greedy list-schedule of one set's matmuls: model subarray busy
    times (merged occupies both row groups of its (R,c) for ~213ns,
    single one; engine issues one MM per ~34ns) and always pick an
    available item, weaving singles between merged to avoid FIFO stalls.
    """
    items = []  # (kind, A, c, j, q)
    for R in range(2):
        r0, r1 = 2 * R, 2 * R + 1
        for j, (k, c, q) in enumerate(rows[r0]):
            if k == "m":
                items.append(("m", R, c, j, q))
        for r in (r0, r1):
            for j, (k, c, q) in enumerate(rows[r]):
                if k == "s":
                    items.append(("s", r, c, j, q))
    pend = list(items)
    busy = {}  # (R, c, rg01) -> free time
    t = 0.0
    order = []
    while pend:
        best_i, best_t = None, None
        for i, it in enumerate(pend):
            kind, A, c, j, q = it
            if kind == "m":
                keys = [(A, c, 0), (A, c, 1)]
            else:
                keys = [(A // 2, c, A % 2)]
            rdy = max([t] + [busy.get(k, 0.0) for k in keys])
            if best_t is None or rdy < best_t - 1e-9:
                best_i, best_t = i, rdy
                if rdy <= t + 1e-9:
                    break
        it = pend.pop(best_i)
        kind, A, c, j, q = it
        if kind == "m":
            keys = [(A, c, 0), (A, c, 1)]
        else:
            keys = [(A // 2, c, A % 2)]
        t = max(best_t, t) + 34.0
        for k in keys:
            busy[k] = max(best_t, t - 34.0) + 213.0
        order.append(it)
    return order


def build_sparse(slot_lists, dt=F16):
    nc = bacc.Bacc("TRN2", target_bir_lowering=False, debug=False)

    plans = build_merge_plan(slot_lists)
    n_sr = np.zeros((NSET, 4), dtype=np.int64)
    for s in range(NSET):
        for r in range(4):
            n_sr[s, r] = len(plans[s][r])
    maxn = int(n_sr.max())
    w_words = int(n_sr.sum()) * 32 * M

    xT = nc.dram_tensor("xT", [P, KO, TOK], dt, kind="ExternalInput")
    w = nc.dram_tensor("w", [w_words], dt, kind="ExternalInput")
    bias = nc.dram_tensor("bias", [P, NSET], F32, kind="ExternalInput")
    outT = nc.dram_tensor("outT", [NSET, P, TOK], F32, kind="ExternalOutput")

    with tile.TileContext(nc) as tc:
        with (
            tc.tile_pool(name="x_pool", bufs=1) as x_pool,
            tc.tile_pool(name="const", bufs=1) as const_pool,
            tc.tile_pool(name="w_pool", bufs=3) as w_pool,
            tc.tile_pool(name="out_pool", bufs=4) as out_pool,
            tc.tile_pool(name="psum", bufs=1, space="PSUM") as psum_pool,
        ):
            XSPLIT = 4
            NXG = KO // XSPLIT
            xts = []
            for g in range(NXG):
                xg = x_pool.tile([P, XSPLIT, TOK], dt, name=f"x{g}", tag=f"x{g}")
                xts.append(xg)

            def emit_x_dma(g):
                nc.sync.dma_start(
                    xts[g][:], xT.ap()[:, g * XSPLIT : (g + 1) * XSPLIT]
                )

            emit_x_dma(0)
            bt = const_pool.tile([P, NSET], F32)
            nc.sync.dma_start(bt[:], bias.ap())

            w_offs = np.zeros((NSET, 4), dtype=np.int64)
            off = 0
            for s in range(NSET):
                for r in range(4):
                    w_offs[s, r] = off
                    off += 32 * int(n_sr[s, r]) * M

            def emit_w_dma(s, wt):
                for r in range(4):
                    n = int(n_sr[s, r])
                    nwords = 32 * n * M
                    o = int(w_offs[s, r])
                    src = w.ap()[o : o + nwords].rearrange("(p f) -> p f", p=32)
                    nc.sync.dma_start(wt[32 * r : 32 * r + 32, : n * M], src)

            # prefetch first two sets' weights ahead of the bulk of x
            pre_wt = {}
            for s in range(min(2, NSET)):
                wt = w_pool.tile([P, maxn * M], dt, tag="w", name="wt")
                emit_w_dma(s, wt)
                pre_wt[s] = wt
            for g in range(1, NXG):
                emit_x_dma(g)

            for s in range(NSET):
                if s in pre_wt:
                    wt = pre_wt[s]
                else:
                    wt = w_pool.tile([P, maxn * M], dt, tag="w", name="wt")
                    emit_w_dma(s, wt)

                ps = [
                    [psum_pool.tile([P, NCHUNK], F32, tag=f"ps_{n}_{r}",
                                    name=f"ps_{n}_{r}")
                     for r in range(4)]
                    for n in range(NT)
                ]

                rows = plans[s]
                order = schedule_items(rows)
                nw = {}
                for kind, A, c, j, q in order:
                    rb = 2 * A if kind == "m" else A
                    nw[(rb, c)] = nw.get((rb, c), 0) + 1

                for n in range(NT):
                    done = {rc: 0 for rc in nw}
                    for kind, A, c, j, q in order:
                        if kind == "m":
                            r_bank, pr, psz = 2 * A, 64 * A, 64
                        else:
                            r_bank, pr, psz = A, 32 * A, 32
                        ko = 0 if q < 0 else q // 4
                        lhsT = wt[pr : pr + psz, j * M : (j + 1) * M]
                        rhs = xts[ko // XSPLIT][
                            pr : pr + psz, ko % XSPLIT,
                            n * NCHUNK : (n + 1) * NCHUNK]
                        rc = (r_bank, c)
                        start = done[rc] == 0
                        done[rc] += 1
                        stop = done[rc] == nw[rc]
                        nc.tensor.matmul(
                            ps[n][r_bank][M * c : M * c + M, :], lhsT, rhs,
                            start=start, stop=stop,
                            tile_position=(pr, M * c),
                        )
                    ot = out_pool.tile([P, NCHUNK], F32, tag="out", name="ot")
                    nc.scalar.activation(
                        ot[:], ps[n][0][:],
                        mybir.ActivationFunctionType.Identity,
                        bias=bt[:, s : s + 1],
                    )
                    for r in range(1, 4):
                        nc.vector.tensor_tensor(
                            ot[:], ot[:], ps[n][r][:], mybir.AluOpType.add
                        )
                    nc.sync.dma_start(
                        outT.ap()[s, :, n * NCHUNK : (n + 1) * NCHUNK], ot[:]
                    )
    nc.compile()
    return nc, n_sr


def build_dense(dt=F16):
    """Dense fallback: [128,128,512] matmuls, K-contiguous, out-group major."""
    NM = OUT // P
    nc = bacc.Bacc("TRN2", target_bir_lowering=False, debug=False)
    xT = nc.dram_tensor("xT", [P, KO, TOK], dt, kind="ExternalInput")
    w = nc.dram_tensor("w", [NM, P, KO, P], dt, kind="ExternalInput")
    bias = nc.dram_tensor("bias", [P, NM], F32, kind="ExternalInput")
    outT = nc.dram_tensor("outT", [NM, P, TOK], F32, kind="ExternalOutput")

    with tile.TileContext(nc) as tc:
        with (
            tc.tile_pool(name="x_pool", bufs=1) as x_pool,
            tc.tile_pool(name="const", bufs=1) as const_pool,
            tc.tile_pool(name="w_pool", bufs=3) as w_pool,
            tc.tile_pool(name="out_pool", bufs=4) as out_pool,
            tc.tile_pool(name="psum", bufs=2, space="PSUM") as psum_pool,
        ):
            xt = x_pool.tile([P, KO, TOK], dt)
            nc.sync.dma_start(xt[:], xT.ap())
            bt = const_pool.tile([P, NM], F32)
            nc.sync.dma_start(bt[:], bias.ap())
            for m in range(NM):
                wt = w_pool.tile([P, KO, P], dt, name="wt")
                nc.sync.dma_start(wt[:], w.ap()[m])
                for n in range(NT):
                    psd = psum_pool.tile([P, NCHUNK], F32, name="psd")
                    for ko in range(KO):
                        nc.tensor.matmul(
                            psd[:], wt[:, ko],
                            xt[:, ko, n * NCHUNK : (n + 1) * NCHUNK],
                            start=(ko == 0), stop=(ko == KO - 1),
                        )
                    ot = out_pool.tile([P, NCHUNK], F32, name="ot")
                    nc.scalar.activation(
                        ot[:], psd[:], mybir.ActivationFunctionType.Identity,
                        bias=bt[:, m : m + 1],
                    )
                    nc.sync.dma_start(
                        outT.ap()[m, :, n * NCHUNK : (n + 1) * NCHUNK], ot[:]
                    )
    nc.compile()
    return nc


# ---------------------------------------------------------------- packing


def group_feats(groups, G):
    return np.concatenate([np.arange(b * BLK, (b + 1) * BLK)
                           for b in groups[G]])


def pack_weights(weight, mask, in_pairs, groups, set_G, slot_lists, n_sr):
    wm = weight.astype(np.float32) * mask
    plans = build_merge_plan(slot_lists)
    total = int(n_sr.sum()) * 32 * M
    out = np.zeros(total, dtype=np.float32)
    off = 0
    for s in range(NSET):
        rows = plans[s]
        for r in range(4):
            n = int(n_sr[s, r])
            assert n == len(rows[r])
            blockbuf = np.zeros((32, n, M), dtype=np.float32)
            for j, (kind, c, q) in enumerate(rows[r]):
                if q < 0:
                    continue
                G = set_G[s][c]
                ofeat = group_feats(groups, G)
                a, b = in_pairs[q]
                ifeat = np.concatenate(
                    [np.arange(a * BLK, (a + 1) * BLK),
                     np.arange(b * BLK, (b + 1) * BLK)]
                )
                blockbuf[:, j, :] = wm[np.ix_(ofeat, ifeat)].T
            nwords = 32 * n * M
            out[off : off + nwords] = blockbuf.reshape(-1)
            off += nwords
    return out.astype(np.float16)


def pack_x_shard(x_shard, in_pairs):
    src_feat = np.empty((P, KO), dtype=np.int64)
    for i in range(NPAIR):
        a, b = in_pairs[i]
        ko, rg = i // 4, i % 4
        src_feat[rg * 32 : rg * 32 + 16, ko] = np.arange(a * BLK, (a + 1) * BLK)
        src_feat[rg * 32 + 16 : rg * 32 + 32, ko] = np.arange(b * BLK,
                                                              (b + 1) * BLK)
    xs = x_shard.astype(np.float16)
    xt = xs.T[src_feat.reshape(-1)].reshape(P, KO, TOK)
    return np.ascontiguousarray(xt)


def pack_bias(bias, groups, set_G):
    bp = np.zeros((P, NSET), dtype=np.float32)
    b = bias.astype(np.float32)
    for s in range(NSET):
        for c in range(2):
            bp[M * c : M * c + M, s] = b[group_feats(groups, set_G[s][c])]
    return bp


def out_feat_map(groups, set_G):
    m = np.empty(OUT, dtype=np.int64)
    for s in range(NSET):
        for c in range(2):
            m[s * P + M * c : s * P + M * c + M] = group_feats(
                groups, set_G[s][c])
    return m


# ---------------------------------------------------------------- entry

_CACHE = {}


def _run_sparse(x, weight, bias, mask, plan):
    nc, in_pairs, groups, set_G, slot_lists, n_sr = plan
    w_flat = pack_weights(weight, mask, in_pairs, groups, set_G,
                          slot_lists, n_sr)
    bias_p = pack_bias(bias, groups, set_G)
    B, S = x.shape[0], x.shape[1]
    xf = np.ascontiguousarray(x.reshape(B * S, IN))
    in_maps = []
    for cidx in range(N_CORES):
        xs = xf[cidx * TOK : (cidx + 1) * TOK]
        in_maps.append({"xT": pack_x_shard(xs, in_pairs), "w": w_flat,
                        "bias": bias_p})
    res = bass_utils.run_bass_kernel_spmd(
        nc, in_maps, core_ids=list(range(N_CORES)))
    fmap = out_feat_map(groups, set_G)
    outs = []
    for cidx in range(N_CORES):
        o = res.results[cidx]["outT"].reshape(OUT, TOK)
        unperm = np.empty_like(o)
        unperm[fmap] = o
        outs.append(unperm.T)
    full = np.concatenate(outs, axis=0)
    return np.ascontiguousarray(full.reshape(B, S, OUT).astype(np.float32))


def _run_dense(x, weight, bias, mask, nc):
    NM = OUT // P
    wm = (weight.astype(np.float32) * mask).astype(np.float16)
    w_packed = np.ascontiguousarray(
        wm.T.reshape(KO, P, NM, P).transpose(2, 1, 0, 3))
    bias_p = np.ascontiguousarray(bias.astype(np.float32).reshape(NM, P).T)
    B, S = x.shape[0], x.shape[1]
    xf = np.ascontiguousarray(x.reshape(B * S, IN))
    in_maps = []
    for cidx in range(N_CORES):
        xs = xf[cidx * TOK : (cidx + 1) * TOK].astype(np.float16)
        xp = np.ascontiguousarray(xs.T.reshape(KO, P, TOK).transpose(1, 0, 2))
        in_maps.append({"xT": xp, "w": w_packed, "bias": bias_p})
    res = bass_utils.run_bass_kernel_spmd(
        nc, in_maps, core_ids=list(range(N_CORES)))
    outs = []
    for cidx in range(N_CORES):
        o = res.results[cidx]["outT"].reshape(OUT, TOK)
        outs.append(o.T)
    full = np.concatenate(outs, axis=0)
    return np.ascontiguousarray(full.reshape(B, S, OUT).astype(np.float32))


def run_traced(inputs):
    """Dev-only traced timing run for test.py; the harness never calls this."""
    x = np.asarray(inputs["x"], dtype=np.float32)
    weight = np.asarray(inputs["weight"], dtype=np.float32)
    bias = np.asarray(inputs["bias"], dtype=np.float32)
    mask = np.asarray(inputs["mask"]).astype(bool)
    kernel(x, weight, bias, mask)  # ensure plan compiled+cached
    key = hash(mask.tobytes())
    kind, plan = _CACHE[key]
    if kind != "sparse":
        return None
    nc, in_pairs, groups, set_G, slot_lists, n_sr = plan
    w_flat = pack_weights(weight, mask, in_pairs, groups, set_G,
                          slot_lists, n_sr)
    bias_p = pack_bias(bias, groups, set_G)
    xf = np.ascontiguousarray(x.reshape(-1, IN))
    in_maps = []
    for cidx in range(N_CORES):
        xs = xf[cidx * TOK : (cidx + 1) * TOK]
        in_maps.append({"xT": pack_x_shard(xs, in_pairs), "w": w_flat,
                        "bias": bias_p})
    return bass_utils.run_bass_kernel_spmd(
        nc, in_maps, core_ids=list(range(N_CORES)), trace=True)


def kernel(x, weight, bias, mask):
    x = np.asarray(x, dtype=np.float32)
    weight = np.asarray(weight, dtype=np.float32)
    bias = np.asarray(bias, dtype=np.float32)
    mask = np.asarray(mask).astype(bool)
    assert x.shape == (4, 2048, IN) and weight.shape == (OUT, IN)

    key = hash(mask.tobytes())
    if key not in _CACHE:
        in_pairs, groups, sc64 = analyze_mask(mask)
        cells = int(sc64.sum())
        if cells <= SPARSE_MAX_CELLS:
            set_G, slot_lists, perm = build_schedule(sc64)
            in_pairs = in_pairs[perm]
            nc, n_sr = build_sparse(slot_lists)
            _CACHE[key] = ("sparse",
                           (nc, in_pairs, groups, set_G, slot_lists, n_sr))
        else:
            _CACHE[key] = ("dense", build_dense())
    kind, plan = _CACHE[key]
    if kind == "sparse":
        return _run_sparse(x, weight, bias, mask, plan)
    return _run_dense(x, weight, bias, mask, plan)



# revision 34
# speedup vs baseline: 1.5397x; 1.5397x over previous
"""Block-sparse linear kernel for Trainium2 (8 NeuronCores, Bass/Tile).

Computes out = x @ (weight*mask).T + bias for
  x [4, 2048, 4096] f32, weight [4096, 4096] f32, mask [4096,4096] bool,
  bias [4096] f32  ->  out [4, 2048, 4096] f32.

Strategy (data-parallel over tokens, 8 cores x 1024 tokens each):
  The 16x16 block mask is coarsened by greedy max-overlap matching into
  supercells of 2 input blocks (K=32) x 4 output blocks (M=64). Only
  nonzero supercells are computed, as [32,64,512] PE-tiled fp16 matmuls
  (fp32 accumulate in PSUM) on 8 concurrent tensor-engine slots
  (4 row groups x 2 column positions). ~3575 supercells vs 16384 dense
  equivalents => ~1.9x over a dense roofline kernel.

  Per core: x resident in SBUF as [128, 32, 1024] fp16 (input pair i at
  partition group i%4, ko i//4). 64 output groups processed in 32 sets of
  2; slot (r, cpos) accumulates into psum bank ps[chunk][r] partitions
  [64*cpos:64*cpos+64). Token chunks (2x512) are processed sequentially so
  chunk-0 psum drains overlap chunk-1 compute. Drain = 1 ScalarE
  activation (bias add) + 3 VectorE adds, then DMA out.

  Falls back to a dense fp16 kernel when the mask is not sparse enough.
"""

import sys

for _p in ("/opt/trn_rl_repo",):
    if _p not in sys.path:
        sys.path.insert(0, _p)

import numpy as np

import concourse.bacc as bacc
import concourse.mybir as mybir
import concourse.tile as tile
from concourse import bass_utils

P = 128
IN = 4096
OUT = 4096
BLK = 16
NB = IN // BLK  # 256 blocks per dim
NPAIR = NB // 2  # 128 input pairs
KO = IN // P  # 32
M = 64  # out-features per supercell
NG = OUT // M  # 64 output groups
NSET = NG // 2  # 32 sets (2 col positions)
N_CORES = 8
TOK = 1024
NCHUNK = 512
NT = TOK // NCHUNK  # 2
F16 = mybir.dt.float16
F32 = mybir.dt.float32

# sparse path wins while 2 * supercells * ~40ns < dense ~450us
SPARSE_MAX_CELLS = 5400


# ---------------------------------------------------------------- matching


def greedy_pair(support):
    """support: [N, D] bool rows. Pair rows maximizing overlap; [N/2, 2]."""
    N = support.shape[0]
    A = support.astype(np.int32)
    O = A @ A.T
    np.fill_diagonal(O, -1)
    pairs = []
    for _ in range(N // 2):
        idx = int(np.argmax(O))
        i, j = divmod(idx, N)
        pairs.append((i, j))
        O[i, :] = -1
        O[:, i] = -1
        O[j, :] = -1
        O[:, j] = -1
    return np.array(pairs, dtype=np.int64)


def analyze_mask(mask):
    """Returns (in_pairs [128,2], groups [64][4 block ids], sc64 [64,128] bool).

    Alternates re-grouping outputs against current input pairs and
    re-pairing inputs against current output groups, keeping the best.
    """
    bm = mask.reshape(NB, BLK, NB, BLK).any(axis=(1, 3))  # [out_blk, in_blk]
    in_pairs = greedy_pair(bm.T)
    best = None
    for _ in range(4):
        # group outputs (4 blocks each) against current input pairs
        bmc = bm[:, in_pairs[:, 0]] | bm[:, in_pairs[:, 1]]  # [256, 128]
        out_pairs = greedy_pair(bmc)
        sc32 = bmc[out_pairs[:, 0]] | bmc[out_pairs[:, 1]]
        rp = greedy_pair(sc32)
        sc64 = sc32[rp[:, 0]] | sc32[rp[:, 1]]  # [64, 128]
        groups = np.array(
            [[out_pairs[a][0], out_pairs[a][1], out_pairs[b][0], out_pairs[b][1]]
             for a, b in rp], dtype=np.int64)
        cells = int(sc64.sum())
        if best is None or cells < best[0]:
            best = (cells, in_pairs.copy(), groups, sc64)
        # re-pair inputs against the output groups
        bg = np.zeros((NG, NB), dtype=bool)  # [group, in_blk]
        for g in range(NG):
            bg[g] = bm[groups[g]].any(axis=0)
        in_pairs = greedy_pair(bg.T)
    _, in_pairs, groups, _ = best

    # refine by alternating hill-climbs: output-block<->group swaps and
    # input-block<->pair swaps, both scored on total nonzero cells
    groups = groups.copy()
    in_pairs = np.array(in_pairs)
    rng = np.random.default_rng(1)

    def refine_groups(ip, groups, iters):
        bp = bm[:, ip[:, 0]] | bm[:, ip[:, 1]]  # [out_blk, pair]
        cnt = np.zeros((NG, NPAIR), dtype=np.int16)
        for g in range(NG):
            cnt[g] = bp[groups[g]].sum(axis=0)
        gi = np.zeros(NB, dtype=np.int64)
        pos = np.zeros(NB, dtype=np.int64)
        for g in range(NG):
            for k in range(4):
                gi[groups[g][k]] = g
                pos[groups[g][k]] = k
        for _ in range(iters):
            u, v = rng.integers(0, NB, 2)
            g1, g2 = gi[u], gi[v]
            if g1 == g2:
                continue
            n1 = cnt[g1] - bp[u] + bp[v]
            n2 = cnt[g2] - bp[v] + bp[u]
            d = (int((n1 > 0).sum()) + int((n2 > 0).sum())
                 - int((cnt[g1] > 0).sum()) - int((cnt[g2] > 0).sum()))
            if d <= 0:
                cnt[g1], cnt[g2] = n1, n2
                k1, k2 = pos[u], pos[v]
                groups[g1][k1], groups[g2][k2] = v, u
                gi[u], gi[v] = g2, g1
                pos[u], pos[v] = k2, k1
        return groups

    def refine_ip(ip, groups, iters):
        bg = np.zeros((NG, NB), dtype=bool)
        for g in range(NG):
            bg[g] = bm[groups[g]].any(axis=0)
        S = bg.T  # [in_blk, group]
        pairs = [tuple(p) for p in ip]

        def pcost(a, b):
            return int((S[a] | S[b]).sum())

        cost = np.array([pcost(a, b) for a, b in pairs])
        pos = np.empty(NB, dtype=np.int64)
        for k, (a, b) in enumerate(pairs):
            pos[a] = pos[b] = k
        for _ in range(iters):
            u, v = rng.integers(0, NB, 2)
            if u == v or pos[u] == pos[v]:
                continue
            ku, kv = pos[u], pos[v]
            au, bu = pairs[ku]
            av, bv = pairs[kv]
            pu = bu if au == u else au
            pv = bv if av == v else av
            c1, c2 = pcost(v, pu), pcost(u, pv)
            d = c1 + c2 - cost[ku] - cost[kv]
            if d <= 0:
                pairs[ku] = (v, pu)
                pairs[kv] = (u, pv)
                cost[ku], cost[kv] = c1, c2
                pos[u], pos[v] = kv, ku
        return np.array(pairs, dtype=np.int64)

    def cells_of(ip, groups):
        bp = bm[:, ip[:, 0]] | bm[:, ip[:, 1]]
        g = np.zeros((NG, NPAIR), dtype=bool)
        for G in range(NG):
            g[G] = bp[groups[G]].any(axis=0)
        return int(g.sum()), g

    groups = refine_groups(in_pairs, groups, 60000)
    best = (*cells_of(in_pairs, groups), in_pairs.copy(), groups.copy())
    for _ in range(5):
        in_pairs = refine_ip(in_pairs, groups, 60000)
        groups = refine_groups(in_pairs, groups, 60000)
        c, g = cells_of(in_pairs, groups)
        if c < best[0]:
            best = (c, g, in_pairs.copy(), groups.copy())
    _, sc64, in_pairs, groups = best
    return in_pairs, groups, sc64


def _rebalance_pairs(sc64, set_G):
    """Permute pair layout positions to even out per-set slot loads."""
    rng = np.random.default_rng(0)
    perm = np.arange(NPAIR)
    use_ids = [[np.nonzero(sc64[set_G[s][c]])[0] for c in range(2)]
               for s in range(NSET)]

    def set_cost(p):
        rg = np.zeros(NPAIR, dtype=np.int64)
        rg[p] = np.arange(NPAIR) % 4
        tot = 0
        for s in range(NSET):
            mx = 1
            for c in range(2):
                ids = use_ids[s][c]
                if len(ids):
                    mx = max(mx, int(np.bincount(rg[ids], minlength=4).max()))
            tot += mx
        return tot

    best = set_cost(perm)
    for _ in range(4000):
        i, j = rng.integers(0, NPAIR, 2)
        if i == j or (i % 4) == (j % 4):
            continue
        perm[[i, j]] = perm[[j, i]]
        c = set_cost(perm)
        if c <= best:
            best = c
        else:
            perm[[i, j]] = perm[[j, i]]
    return perm


def build_schedule(sc64):
    counts = sc64.sum(axis=1)
    order = np.argsort(-counts)
    set_G = [[int(order[s * 2 + c]) for c in range(2)] for s in range(NSET)]
    perm = _rebalance_pairs(sc64, set_G)
    pair_pos = np.empty(NPAIR, dtype=np.int64)
    pair_pos[perm] = np.arange(NPAIR)
    slot_lists = []
    for s in range(NSET):
        rows = []
        for r in range(4):
            cols = []
            for c in range(2):
                G = set_G[s][c]
                cs = sorted(int(pair_pos[i]) for i in np.nonzero(sc64[G])[0]
                            if pair_pos[i] % 4 == r)
                cols.append(cs)
            rows.append(cols)
        slot_lists.append(rows)
    return set_G, slot_lists, perm


# ---------------------------------------------------------------- device


def build_sparse(slot_lists, dt=F16):
    nc = bacc.Bacc("TRN2", target_bir_lowering=False, debug=False)

    n_sr = np.zeros((NSET, 4), dtype=np.int64)
    for s in range(NSET):
        for r in range(4):
            n_sr[s, r] = sum(max(1, len(slot_lists[s][r][c])) for c in range(2))
    maxn = int(n_sr.max())
    w_words = int(n_sr.sum()) * 32 * M

    xT = nc.dram_tensor("xT", [P, KO, TOK], dt, kind="ExternalInput")
    w = nc.dram_tensor("w", [w_words], dt, kind="ExternalInput")
    bias = nc.dram_tensor("bias", [P, NSET], F32, kind="ExternalInput")
    outT = nc.dram_tensor("outT", [NSET, P, TOK], F32, kind="ExternalOutput")

    with tile.TileContext(nc) as tc:
        with (
            tc.tile_pool(name="x_pool", bufs=1) as x_pool,
            tc.tile_pool(name="const", bufs=1) as const_pool,
            tc.tile_pool(name="w_pool", bufs=6) as w_pool,
            tc.tile_pool(name="out_pool", bufs=4) as out_pool,
            tc.tile_pool(name="psum", bufs=1, space="PSUM") as psum_pool,
        ):
            XSPLIT = 4
            NXG = KO // XSPLIT
            xts = []
            for g in range(NXG):
                xg = x_pool.tile([P, XSPLIT, TOK], dt, name=f"x{g}", tag=f"x{g}")
                xts.append(xg)

            def emit_x_dma(g):
                nc.sync.dma_start(
                    xts[g][:], xT.ap()[:, g * XSPLIT : (g + 1) * XSPLIT]
                )

            emit_x_dma(0)
            bt = const_pool.tile([P, NSET], F32)
            nc.sync.dma_start(bt[:], bias.ap())

            w_offs = np.zeros((NSET, 4), dtype=np.int64)
            off = 0
            for s in range(NSET):
                for r in range(4):
                    w_offs[s, r] = off
                    off += 32 * int(n_sr[s, r]) * M

            def emit_w_dma(s, wt):
                for r in range(4):
                    n = int(n_sr[s, r])
                    nwords = 32 * n * M
                    o = int(w_offs[s, r])
                    src = w.ap()[o : o + nwords].rearrange("(p f) -> p f", p=32)
                    nc.sync.dma_start(wt[32 * r : 32 * r + 32, : n * M], src)

            # prefetch first two sets' weights ahead of the bulk of x
            pre_wt = {}
            for s in range(min(2, NSET)):
                wt = w_pool.tile([P, maxn * M], dt, tag="w", name="wt")
                emit_w_dma(s, wt)
                pre_wt[s] = wt
            for g in range(1, NXG):
                emit_x_dma(g)

            for s in range(NSET):
                if s in pre_wt:
                    wt = pre_wt[s]
                else:
                    wt = w_pool.tile([P, maxn * M], dt, tag="w", name="wt")
                    emit_w_dma(s, wt)

                ps = [
                    [psum_pool.tile([P, NCHUNK], F32, tag=f"ps_{n}_{r}",
                                    name=f"ps_{n}_{r}")
                     for r in range(4)]
                    for n in range(NT)
                ]

                items = {}
                for r in range(4):
                    for c in range(2):
                        lst = slot_lists[s][r][c]
                        items[(r, c)] = lst if lst else [-1]
                slot_widx = {}
                for r in range(4):
                    k = 0
                    for c in range(2):
                        slot_widx[(r, c)] = k
                        k += len(items[(r, c)])
                nsteps = max(len(items[(r, c)])
                             for r in range(4) for c in range(2))

                for n in range(NT):
                    for st in range(nsteps):
                        for r in range(4):
                            for c in range(2):
                                lst = items[(r, c)]
                                if st >= len(lst):
                                    continue
                                ci = lst[st]
                                wi = slot_widx[(r, c)] + st
                                lhsT = wt[32 * r : 32 * r + 32,
                                          wi * M : (wi + 1) * M]
                                start = st == 0
                                stop = st == len(lst) - 1
                                if ci < 0:
                                    ko, rg = 0, r
                                else:
                                    ko, rg = ci // 4, ci % 4
                                rhs = xts[ko // XSPLIT][
                                    32 * rg : 32 * rg + 32, ko % XSPLIT,
                                    n * NCHUNK : (n + 1) * NCHUNK]
                                nc.tensor.matmul(
                                    ps[n][r][M * c : M * c + M, :], lhsT, rhs,
                                    start=start, stop=stop,
                                    tile_position=(32 * r, M * c),
                                )
                    ot = out_pool.tile([P, NCHUNK], F32, tag="out", name="ot")
                    nc.scalar.activation(
                        ot[:], ps[n][0][:],
                        mybir.ActivationFunctionType.Identity,
                        bias=bt[:, s : s + 1],
                    )
                    for r in range(1, 4):
                        nc.vector.tensor_tensor(
                            ot[:], ot[:], ps[n][r][:], mybir.AluOpType.add
                        )
                    nc.sync.dma_start(
                        outT.ap()[s, :, n * NCHUNK : (n + 1) * NCHUNK], ot[:]
                    )
    nc.compile()
    return nc, n_sr


def build_dense(dt=F16):
    """Dense fallback: [128,128,512] matmuls, K-contiguous, out-group major."""
    NM = OUT // P
    nc = bacc.Bacc("TRN2", target_bir_lowering=False, debug=False)
    xT = nc.dram_tensor("xT", [P, KO, TOK], dt, kind="ExternalInput")
    w = nc.dram_tensor("w", [NM, P, KO, P], dt, kind="ExternalInput")
    bias = nc.dram_tensor("bias", [P, NM], F32, kind="ExternalInput")
    outT = nc.dram_tensor("outT", [NM, P, TOK], F32, kind="ExternalOutput")

    with tile.TileContext(nc) as tc:
        with (
            tc.tile_pool(name="x_pool", bufs=1) as x_pool,
            tc.tile_pool(name="const", bufs=1) as const_pool,
            tc.tile_pool(name="w_pool", bufs=6) as w_pool,
            tc.tile_pool(name="out_pool", bufs=4) as out_pool,
            tc.tile_pool(name="psum", bufs=2, space="PSUM") as psum_pool,
        ):
            xt = x_pool.tile([P, KO, TOK], dt)
            nc.sync.dma_start(xt[:], xT.ap())
            bt = const_pool.tile([P, NM], F32)
            nc.sync.dma_start(bt[:], bias.ap())
            for m in range(NM):
                wt = w_pool.tile([P, KO, P], dt, name="wt")
                nc.sync.dma_start(wt[:], w.ap()[m])
                for n in range(NT):
                    psd = psum_pool.tile([P, NCHUNK], F32, name="psd")
                    for ko in range(KO):
                        nc.tensor.matmul(
                            psd[:], wt[:, ko],
                            xt[:, ko, n * NCHUNK : (n + 1) * NCHUNK],
                            start=(ko == 0), stop=(ko == KO - 1),
                        )
                    ot = out_pool.tile([P, NCHUNK], F32, name="ot")
                    nc.scalar.activation(
                        ot[:], psd[:], mybir.ActivationFunctionType.Identity,
                        bias=bt[:, m : m + 1],
                    )
                    nc.sync.dma_start(
                        outT.ap()[m, :, n * NCHUNK : (n + 1) * NCHUNK], ot[:]
                    )
    nc.compile()
    return nc


# ---------------------------------------------------------------- packing


def group_feats(groups, G):
    return np.concatenate([np.arange(b * BLK, (b + 1) * BLK)
                           for b in groups[G]])


def pack_weights(weight, mask, in_pairs, groups, set_G, slot_lists, n_sr):
    wm = weight.astype(np.float32) * mask
    total = int(n_sr.sum()) * 32 * M
    out = np.zeros(total, dtype=np.float32)
    off = 0
    for s in range(NSET):
        for r in range(4):
            n = int(n_sr[s, r])
            blockbuf = np.zeros((32, n, M), dtype=np.float32)
            k = 0
            for c in range(2):
                G = set_G[s][c]
                ofeat = group_feats(groups, G)
                lst = slot_lists[s][r][c]
                if not lst:
                    k += 1
                    continue
                for ci in lst:
                    a, b = in_pairs[ci]
                    ifeat = np.concatenate(
                        [np.arange(a * BLK, (a + 1) * BLK),
                         np.arange(b * BLK, (b + 1) * BLK)]
                    )
                    blockbuf[:, k, :] = wm[np.ix_(ofeat, ifeat)].T
                    k += 1
            assert k == n
            nwords = 32 * n * M
            out[off : off + nwords] = blockbuf.reshape(-1)
            off += nwords
    return out.astype(np.float16)


def pack_x_shard(x_shard, in_pairs):
    src_feat = np.empty((P, KO), dtype=np.int64)
    for i in range(NPAIR):
        a, b = in_pairs[i]
        ko, rg = i // 4, i % 4
        src_feat[rg * 32 : rg * 32 + 16, ko] = np.arange(a * BLK, (a + 1) * BLK)
        src_feat[rg * 32 + 16 : rg * 32 + 32, ko] = np.arange(b * BLK,
                                                              (b + 1) * BLK)
    xs = x_shard.astype(np.float16)
    xt = xs.T[src_feat.reshape(-1)].reshape(P, KO, TOK)
    return np.ascontiguousarray(xt)


def pack_bias(bias, groups, set_G):
    bp = np.zeros((P, NSET), dtype=np.float32)
    b = bias.astype(np.float32)
    for s in range(NSET):
        for c in range(2):
            bp[M * c : M * c + M, s] = b[group_feats(groups, set_G[s][c])]
    return bp


def out_feat_map(groups, set_G):
    m = np.empty(OUT, dtype=np.int64)
    for s in range(NSET):
        for c in range(2):
            m[s * P + M * c : s * P + M * c + M] = group_feats(
                groups, set_G[s][c])
    return m


# ---------------------------------------------------------------- entry

_CACHE = {}


def _run_sparse(x, weight, bias, mask, plan):
    nc, in_pairs, groups, set_G, slot_lists, n_sr = plan
    w_flat = pack_weights(weight, mask, in_pairs, groups, set_G,
                          slot_lists, n_sr)
    bias_p = pack_bias(bias, groups, set_G)
    B, S = x.shape[0], x.shape[1]
    xf = np.ascontiguousarray(x.reshape(B * S, IN))
    in_maps = []
    for cidx in range(N_CORES):
        xs = xf[cidx * TOK : (cidx + 1) * TOK]
        in_maps.append({"xT": pack_x_shard(xs, in_pairs), "w": w_flat,
                        "bias": bias_p})
    res = bass_utils.run_bass_kernel_spmd(
        nc, in_maps, core_ids=list(range(N_CORES)))
    fmap = out_feat_map(groups, set_G)
    outs = []
    for cidx in range(N_CORES):
        o = res.results[cidx]["outT"].reshape(OUT, TOK)
        unperm = np.empty_like(o)
        unperm[fmap] = o
        outs.append(unperm.T)
    full = np.concatenate(outs, axis=0)
    return np.ascontiguousarray(full.reshape(B, S, OUT).astype(np.float32))


def _run_dense(x, weight, bias, mask, nc):
    NM = OUT // P
    wm = (weight.astype(np.float32) * mask).astype(np.float16)
    w_packed = np.ascontiguousarray(
        wm.T.reshape(KO, P, NM, P).transpose(2, 1, 0, 3))
    bias_p = np.ascontiguousarray(bias.astype(np.float32).reshape(NM, P).T)
    B, S = x.shape[0], x.shape[1]
    xf = np.ascontiguousarray(x.reshape(B * S, IN))
    in_maps = []
    for cidx in range(N_CORES):
        xs = xf[cidx * TOK : (cidx + 1) * TOK].astype(np.float16)
        xp = np.ascontiguousarray(xs.T.reshape(KO, P, TOK).transpose(1, 0, 2))
        in_maps.append({"xT": xp, "w": w_packed, "bias": bias_p})
    res = bass_utils.run_bass_kernel_spmd(
        nc, in_maps, core_ids=list(range(N_CORES)))
    outs = []
    for cidx in range(N_CORES):
        o = res.results[cidx]["outT"].reshape(OUT, TOK)
        outs.append(o.T)
    full = np.concatenate(outs, axis=0)
    return np.ascontiguousarray(full.reshape(B, S, OUT).astype(np.float32))


def run_traced(inputs):
    """Dev-only traced timing run for test.py; the harness never calls this."""
    x = np.asarray(inputs["x"], dtype=np.float32)
    weight = np.asarray(inputs["weight"], dtype=np.float32)
    bias = np.asarray(inputs["bias"], dtype=np.float32)
    mask = np.asarray(inputs["mask"]).astype(bool)
    kernel(x, weight, bias, mask)  # ensure plan compiled+cached
    key = hash(mask.tobytes())
    kind, plan = _CACHE[key]
    if kind != "sparse":
        return None
    nc, in_pairs, groups, set_G, slot_lists, n_sr = plan
    w_flat = pack_weights(weight, mask, in_pairs, groups, set_G,
                          slot_lists, n_sr)
    bias_p = pack_bias(bias, groups, set_G)
    xf = np.ascontiguousarray(x.reshape(-1, IN))
    in_maps = []
    for cidx in range(N_CORES):
        xs = xf[cidx * TOK : (cidx + 1) * TOK]
        in_maps.append({"xT": pack_x_shard(xs, in_pairs), "w": w_flat,
                        "bias": bias_p})
    return bass_utils.run_bass_kernel_spmd(
        nc, in_maps, core_ids=list(range(N_CORES)), trace=True)


def kernel(x, weight, bias, mask):
    x = np.asarray(x, dtype=np.float32)
    weight = np.asarray(weight, dtype=np.float32)
    bias = np.asarray(bias, dtype=np.float32)
    mask = np.asarray(mask).astype(bool)
    assert x.shape == (4, 2048, IN) and weight.shape == (OUT, IN)

    key = hash(mask.tobytes())
    if key not in _CACHE:
        in_pairs, groups, sc64 = analyze_mask(mask)
        cells = int(sc64.sum())
        if cells <= SPARSE_MAX_CELLS:
            set_G, slot_lists, perm = build_schedule(sc64)
            in_pairs = in_pairs[perm]
            nc, n_sr = build_sparse(slot_lists)
            _CACHE[key] = ("sparse",
                           (nc, in_pairs, groups, set_G, slot_lists, n_sr))
        else:
            _CACHE[key] = ("dense", build_dense())
    kind, plan = _CACHE[key]
    if kind == "sparse":
        return _run_sparse(x, weight, bias, mask, plan)
    return _run_dense(x, weight, bias, mask, plan)



# revision 35
# speedup vs baseline: 1.5455x; 1.0038x over previous
"""Block-sparse linear kernel for Trainium2 (8 NeuronCores, Bass/Tile).

Computes out = x @ (weight*mask).T + bias for
  x [4, 2048, 4096] f32, weight [4096, 4096] f32, mask [4096,4096] bool,
  bias [4096] f32  ->  out [4, 2048, 4096] f32.

Strategy (data-parallel over tokens, 8 cores x 1024 tokens each):
  The 16x16 block mask is coarsened by greedy max-overlap matching into
  supercells of 2 input blocks (K=32) x 4 output blocks (M=64). Only
  nonzero supercells are computed, as [32,64,512] PE-tiled fp16 matmuls
  (fp32 accumulate in PSUM) on 8 concurrent tensor-engine slots
  (4 row groups x 2 column positions). ~3575 supercells vs 16384 dense
  equivalents => ~1.9x over a dense roofline kernel.

  Per core: x resident in SBUF as [128, 32, 1024] fp16 (input pair i at
  partition group i%4, ko i//4). 64 output groups processed in 32 sets of
  2; slot (r, cpos) accumulates into psum bank ps[chunk][r] partitions
  [64*cpos:64*cpos+64). Token chunks (2x512) are processed sequentially so
  chunk-0 psum drains overlap chunk-1 compute. Drain = 1 ScalarE
  activation (bias add) + 3 VectorE adds, then DMA out.

  Falls back to a dense fp16 kernel when the mask is not sparse enough.
"""

import sys

for _p in ("/opt/trn_rl_repo",):
    if _p not in sys.path:
        sys.path.insert(0, _p)

import numpy as np

import concourse.bacc as bacc
import concourse.mybir as mybir
import concourse.tile as tile
from concourse import bass_utils

P = 128
IN = 4096
OUT = 4096
BLK = 16
NB = IN // BLK  # 256 blocks per dim
NPAIR = NB // 2  # 128 input pairs
KO = IN // P  # 32
M = 64  # out-features per supercell
NG = OUT // M  # 64 output groups
NSET = NG // 2  # 32 sets (2 col positions)
N_CORES = 8
TOK = 1024
NCHUNK = 512
NT = TOK // NCHUNK  # 2
F16 = mybir.dt.float16
F32 = mybir.dt.float32

# sparse path wins while 2 * supercells * ~40ns < dense ~450us
SPARSE_MAX_CELLS = 5400


# ---------------------------------------------------------------- matching


def greedy_pair(support):
    """support: [N, D] bool rows. Pair rows maximizing overlap; [N/2, 2]."""
    N = support.shape[0]
    A = support.astype(np.int32)
    O = A @ A.T
    np.fill_diagonal(O, -1)
    pairs = []
    for _ in range(N // 2):
        idx = int(np.argmax(O))
        i, j = divmod(idx, N)
        pairs.append((i, j))
        O[i, :] = -1
        O[:, i] = -1
        O[j, :] = -1
        O[:, j] = -1
    return np.array(pairs, dtype=np.int64)


def analyze_mask(mask):
    """Returns (in_pairs [128,2], groups [64][4 block ids], sc64 [64,128] bool).

    Alternates re-grouping outputs against current input pairs and
    re-pairing inputs against current output groups, keeping the best.
    """
    bm = mask.reshape(NB, BLK, NB, BLK).any(axis=(1, 3))  # [out_blk, in_blk]
    in_pairs = greedy_pair(bm.T)
    best = None
    for _ in range(4):
        # group outputs (4 blocks each) against current input pairs
        bmc = bm[:, in_pairs[:, 0]] | bm[:, in_pairs[:, 1]]  # [256, 128]
        out_pairs = greedy_pair(bmc)
        sc32 = bmc[out_pairs[:, 0]] | bmc[out_pairs[:, 1]]
        rp = greedy_pair(sc32)
        sc64 = sc32[rp[:, 0]] | sc32[rp[:, 1]]  # [64, 128]
        groups = np.array(
            [[out_pairs[a][0], out_pairs[a][1], out_pairs[b][0], out_pairs[b][1]]
             for a, b in rp], dtype=np.int64)
        cells = int(sc64.sum())
        if best is None or cells < best[0]:
            best = (cells, in_pairs.copy(), groups, sc64)
        # re-pair inputs against the output groups
        bg = np.zeros((NG, NB), dtype=bool)  # [group, in_blk]
        for g in range(NG):
            bg[g] = bm[groups[g]].any(axis=0)
        in_pairs = greedy_pair(bg.T)
    _, in_pairs, groups, _ = best

    # refine by alternating hill-climbs: output-block<->group swaps and
    # input-block<->pair swaps, both scored on total nonzero cells
    groups = groups.copy()
    in_pairs = np.array(in_pairs)
    rng = np.random.default_rng(1)

    def refine_groups(ip, groups, iters):
        bp = bm[:, ip[:, 0]] | bm[:, ip[:, 1]]  # [out_blk, pair]
        cnt = np.zeros((NG, NPAIR), dtype=np.int16)
        for g in range(NG):
            cnt[g] = bp[groups[g]].sum(axis=0)
        gi = np.zeros(NB, dtype=np.int64)
        pos = np.zeros(NB, dtype=np.int64)
        for g in range(NG):
            for k in range(4):
                gi[groups[g][k]] = g
                pos[groups[g][k]] = k
        for _ in range(iters):
            u, v = rng.integers(0, NB, 2)
            g1, g2 = gi[u], gi[v]
            if g1 == g2:
                continue
            n1 = cnt[g1] - bp[u] + bp[v]
            n2 = cnt[g2] - bp[v] + bp[u]
            d = (int((n1 > 0).sum()) + int((n2 > 0).sum())
                 - int((cnt[g1] > 0).sum()) - int((cnt[g2] > 0).sum()))
            if d <= 0:
                cnt[g1], cnt[g2] = n1, n2
                k1, k2 = pos[u], pos[v]
                groups[g1][k1], groups[g2][k2] = v, u
                gi[u], gi[v] = g2, g1
                pos[u], pos[v] = k2, k1
        return groups

    def refine_ip(ip, groups, iters):
        bg = np.zeros((NG, NB), dtype=bool)
        for g in range(NG):
            bg[g] = bm[groups[g]].any(axis=0)
        S = bg.T  # [in_blk, group]
        pairs = [tuple(p) for p in ip]

        def pcost(a, b):
            return int((S[a] | S[b]).sum())

        cost = np.array([pcost(a, b) for a, b in pairs])
        pos = np.empty(NB, dtype=np.int64)
        for k, (a, b) in enumerate(pairs):
            pos[a] = pos[b] = k
        for _ in range(iters):
            u, v = rng.integers(0, NB, 2)
            if u == v or pos[u] == pos[v]:
                continue
            ku, kv = pos[u], pos[v]
            au, bu = pairs[ku]
            av, bv = pairs[kv]
            pu = bu if au == u else au
            pv = bv if av == v else av
            c1, c2 = pcost(v, pu), pcost(u, pv)
            d = c1 + c2 - cost[ku] - cost[kv]
            if d <= 0:
                pairs[ku] = (v, pu)
                pairs[kv] = (u, pv)
                cost[ku], cost[kv] = c1, c2
                pos[u], pos[v] = kv, ku
        return np.array(pairs, dtype=np.int64)

    def cells_of(ip, groups):
        bp = bm[:, ip[:, 0]] | bm[:, ip[:, 1]]
        g = np.zeros((NG, NPAIR), dtype=bool)
        for G in range(NG):
            g[G] = bp[groups[G]].any(axis=0)
        return int(g.sum()), g

    groups = refine_groups(in_pairs, groups, 60000)
    best = (*cells_of(in_pairs, groups), in_pairs.copy(), groups.copy())
    for _ in range(5):
        in_pairs = refine_ip(in_pairs, groups, 60000)
        groups = refine_groups(in_pairs, groups, 60000)
        c, g = cells_of(in_pairs, groups)
        if c < best[0]:
            best = (c, g, in_pairs.copy(), groups.copy())
    _, sc64, in_pairs, groups = best
    return in_pairs, groups, sc64


def _rebalance_pairs(sc64, set_G):
    """Permute pair layout positions to even out per-set slot loads."""
    rng = np.random.default_rng(0)
    perm = np.arange(NPAIR)
    use_ids = [[np.nonzero(sc64[set_G[s][c]])[0] for c in range(2)]
               for s in range(NSET)]

    def set_cost(p):
        rg = np.zeros(NPAIR, dtype=np.int64)
        rg[p] = np.arange(NPAIR) % 4
        tot = 0
        for s in range(NSET):
            mx = 1
            for c in range(2):
                ids = use_ids[s][c]
                if len(ids):
                    mx = max(mx, int(np.bincount(rg[ids], minlength=4).max()))
            tot += mx
        return tot

    best = set_cost(perm)
    for _ in range(4000):
        i, j = rng.integers(0, NPAIR, 2)
        if i == j or (i % 4) == (j % 4):
            continue
        perm[[i, j]] = perm[[j, i]]
        c = set_cost(perm)
        if c <= best:
            best = c
        else:
            perm[[i, j]] = perm[[j, i]]
    return perm


def build_schedule(sc64):
    counts = sc64.sum(axis=1)
    order = np.argsort(-counts)
    set_G = [[int(order[s * 2 + c]) for c in range(2)] for s in range(NSET)]
    perm = _rebalance_pairs(sc64, set_G)
    pair_pos = np.empty(NPAIR, dtype=np.int64)
    pair_pos[perm] = np.arange(NPAIR)
    slot_lists = []
    for s in range(NSET):
        rows = []
        for r in range(4):
            cols = []
            for c in range(2):
                G = set_G[s][c]
                cs = sorted(int(pair_pos[i]) for i in np.nonzero(sc64[G])[0]
                            if pair_pos[i] % 4 == r)
                cols.append(cs)
            rows.append(cols)
        slot_lists.append(rows)
    return set_G, slot_lists, perm


# ---------------------------------------------------------------- device


def build_sparse(slot_lists, dt=F16):
    nc = bacc.Bacc("TRN2", target_bir_lowering=False, debug=False)

    n_sr = np.zeros((NSET, 4), dtype=np.int64)
    for s in range(NSET):
        for r in range(4):
            n_sr[s, r] = sum(max(1, len(slot_lists[s][r][c])) for c in range(2))
    maxn = int(n_sr.max())
    w_words = int(n_sr.sum()) * 32 * M

    xT = nc.dram_tensor("xT", [P, KO, TOK], dt, kind="ExternalInput")
    w = nc.dram_tensor("w", [w_words], dt, kind="ExternalInput")
    bias = nc.dram_tensor("bias", [P, NSET], F32, kind="ExternalInput")
    outT = nc.dram_tensor("outT", [NSET, P, TOK], F32, kind="ExternalOutput")

    with tile.TileContext(nc) as tc:
        with (
            tc.tile_pool(name="x_pool", bufs=1) as x_pool,
            tc.tile_pool(name="const", bufs=1) as const_pool,
            tc.tile_pool(name="w_pool", bufs=8) as w_pool,
            tc.tile_pool(name="out_pool", bufs=6) as out_pool,
            tc.tile_pool(name="psum", bufs=1, space="PSUM") as psum_pool,
        ):
            XSPLIT = 4
            NXG = KO // XSPLIT
            xts = []
            for g in range(NXG):
                xg = x_pool.tile([P, XSPLIT, TOK], dt, name=f"x{g}", tag=f"x{g}")
                xts.append(xg)

            def emit_x_dma(g):
                nc.sync.dma_start(
                    xts[g][:], xT.ap()[:, g * XSPLIT : (g + 1) * XSPLIT]
                )

            emit_x_dma(0)
            bt = const_pool.tile([P, NSET], F32)
            nc.sync.dma_start(bt[:], bias.ap())

            w_offs = np.zeros((NSET, 4), dtype=np.int64)
            off = 0
            for s in range(NSET):
                for r in range(4):
                    w_offs[s, r] = off
                    off += 32 * int(n_sr[s, r]) * M

            def emit_w_dma(s, wt):
                for r in range(4):
                    n = int(n_sr[s, r])
                    nwords = 32 * n * M
                    o = int(w_offs[s, r])
                    src = w.ap()[o : o + nwords].rearrange("(p f) -> p f", p=32)
                    nc.sync.dma_start(wt[32 * r : 32 * r + 32, : n * M], src)

            # prefetch first two sets' weights ahead of the bulk of x
            pre_wt = {}
            for s in range(min(2, NSET)):
                wt = w_pool.tile([P, maxn * M], dt, tag="w", name="wt")
                emit_w_dma(s, wt)
                pre_wt[s] = wt
            for g in range(1, NXG):
                emit_x_dma(g)

            for s in range(NSET):
                if s in pre_wt:
                    wt = pre_wt[s]
                else:
                    wt = w_pool.tile([P, maxn * M], dt, tag="w", name="wt")
                    emit_w_dma(s, wt)

                ps = [
                    [psum_pool.tile([P, NCHUNK], F32, tag=f"ps_{n}_{r}",
                                    name=f"ps_{n}_{r}")
                     for r in range(4)]
                    for n in range(NT)
                ]

                items = {}
                for r in range(4):
                    for c in range(2):
                        lst = slot_lists[s][r][c]
                        items[(r, c)] = lst if lst else [-1]
                slot_widx = {}
                for r in range(4):
                    k = 0
                    for c in range(2):
                        slot_widx[(r, c)] = k
                        k += len(items[(r, c)])
                nsteps = max(len(items[(r, c)])
                             for r in range(4) for c in range(2))

                for n in range(NT):
                    for st in range(nsteps):
                        for r in range(4):
                            for c in range(2):
                                lst = items[(r, c)]
                                if st >= len(lst):
                                    continue
                                ci = lst[st]
                                wi = slot_widx[(r, c)] + st
                                lhsT = wt[32 * r : 32 * r + 32,
                                          wi * M : (wi + 1) * M]
                                start = st == 0
                                stop = st == len(lst) - 1
                                if ci < 0:
                                    ko, rg = 0, r
                                else:
                                    ko, rg = ci // 4, ci % 4
                                rhs = xts[ko // XSPLIT][
                                    32 * rg : 32 * rg + 32, ko % XSPLIT,
                                    n * NCHUNK : (n + 1) * NCHUNK]
                                nc.tensor.matmul(
                                    ps[n][r][M * c : M * c + M, :], lhsT, rhs,
                                    start=start, stop=stop,
                                    tile_position=(32 * r, M * c),
                                )
                    ot = out_pool.tile([P, NCHUNK], F32, tag="out", name="ot")
                    nc.scalar.activation(
                        ot[:], ps[n][0][:],
                        mybir.ActivationFunctionType.Identity,
                        bias=bt[:, s : s + 1],
                    )
                    for r in range(1, 4):
                        nc.vector.tensor_tensor(
                            ot[:], ot[:], ps[n][r][:], mybir.AluOpType.add
                        )
                    nc.sync.dma_start(
                        outT.ap()[s, :, n * NCHUNK : (n + 1) * NCHUNK], ot[:]
                    )
    nc.compile()
    return nc, n_sr


def build_dense(dt=F16):
    """Dense fallback: [128,128,512] matmuls, K-contiguous, out-group major."""
    NM = OUT // P
    nc = bacc.Bacc("TRN2", target_bir_lowering=False, debug=False)
    xT = nc.dram_tensor("xT", [P, KO, TOK], dt, kind="ExternalInput")
    w = nc.dram_tensor("w", [NM, P, KO, P], dt, kind="ExternalInput")
    bias = nc.dram_tensor("bias", [P, NM], F32, kind="ExternalInput")
    outT = nc.dram_tensor("outT", [NM, P, TOK], F32, kind="ExternalOutput")

    with tile.TileContext(nc) as tc:
        with (
            tc.tile_pool(name="x_pool", bufs=1) as x_pool,
            tc.tile_pool(name="const", bufs=1) as const_pool,
            tc.tile_pool(name="w_pool", bufs=8) as w_pool,
            tc.tile_pool(name="out_pool", bufs=6) as out_pool,
            tc.tile_pool(name="psum", bufs=2, space="PSUM") as psum_pool,
        ):
            xt = x_pool.tile([P, KO, TOK], dt)
            nc.sync.dma_start(xt[:], xT.ap())
            bt = const_pool.tile([P, NM], F32)
            nc.sync.dma_start(bt[:], bias.ap())
            for m in range(NM):
                wt = w_pool.tile([P, KO, P], dt, name="wt")
                nc.sync.dma_start(wt[:], w.ap()[m])
                for n in range(NT):
                    psd = psum_pool.tile([P, NCHUNK], F32, name="psd")
                    for ko in range(KO):
                        nc.tensor.matmul(
                            psd[:], wt[:, ko],
                            xt[:, ko, n * NCHUNK : (n + 1) * NCHUNK],
                            start=(ko == 0), stop=(ko == KO - 1),
                        )
                    ot = out_pool.tile([P, NCHUNK], F32, name="ot")
                    nc.scalar.activation(
                        ot[:], psd[:], mybir.ActivationFunctionType.Identity,
                        bias=bt[:, m : m + 1],
                    )
                    nc.sync.dma_start(
                        outT.ap()[m, :, n * NCHUNK : (n + 1) * NCHUNK], ot[:]
                    )
    nc.compile()
    return nc


# ---------------------------------------------------------------- packing


def group_feats(groups, G):
    return np.concatenate([np.arange(b * BLK, (b + 1) * BLK)
                           for b in groups[G]])


def pack_weights(weight, mask, in_pairs, groups, set_G, slot_lists, n_sr):
    wm = weight.astype(np.float32) * mask
    total = int(n_sr.sum()) * 32 * M
    out = np.zeros(total, dtype=np.float32)
    off = 0
    for s in range(NSET):
        for r in range(4):
            n = int(n_sr[s, r])
            blockbuf = np.zeros((32, n, M), dtype=np.float32)
            k = 0
            for c in range(2):
                G = set_G[s][c]
                ofeat = group_feats(groups, G)
                lst = slot_lists[s][r][c]
                if not lst:
                    k += 1
                    continue
                for ci in lst:
                    a, b = in_pairs[ci]
                    ifeat = np.concatenate(
                        [np.arange(a * BLK, (a + 1) * BLK),
                         np.arange(b * BLK, (b + 1) * BLK)]
                    )
                    blockbuf[:, k, :] = wm[np.ix_(ofeat, ifeat)].T
                    k += 1
            assert k == n
            nwords = 32 * n * M
            out[off : off + nwords] = blockbuf.reshape(-1)
            off += nwords
    return out.astype(np.float16)


def pack_x_shard(x_shard, in_pairs):
    src_feat = np.empty((P, KO), dtype=np.int64)
    for i in range(NPAIR):
        a, b = in_pairs[i]
        ko, rg = i // 4, i % 4
        src_feat[rg * 32 : rg * 32 + 16, ko] = np.arange(a * BLK, (a + 1) * BLK)
        src_feat[rg * 32 + 16 : rg * 32 + 32, ko] = np.arange(b * BLK,
                                                              (b + 1) * BLK)
    xs = x_shard.astype(np.float16)
    xt = xs.T[src_feat.reshape(-1)].reshape(P, KO, TOK)
    return np.ascontiguousarray(xt)


def pack_bias(bias, groups, set_G):
    bp = np.zeros((P, NSET), dtype=np.float32)
    b = bias.astype(np.float32)
    for s in range(NSET):
        for c in range(2):
            bp[M * c : M * c + M, s] = b[group_feats(groups, set_G[s][c])]
    return bp


def out_feat_map(groups, set_G):
    m = np.empty(OUT, dtype=np.int64)
    for s in range(NSET):
        for c in range(2):
            m[s * P + M * c : s * P + M * c + M] = group_feats(
                groups, set_G[s][c])
    return m


# ---------------------------------------------------------------- entry

_CACHE = {}


def _run_sparse(x, weight, bias, mask, plan):
    nc, in_pairs, groups, set_G, slot_lists, n_sr = plan
    w_flat = pack_weights(weight, mask, in_pairs, groups, set_G,
                          slot_lists, n_sr)
    bias_p = pack_bias(bias, groups, set_G)
    B, S = x.shape[0], x.shape[1]
    xf = np.ascontiguousarray(x.reshape(B * S, IN))
    in_maps = []
    for cidx in range(N_CORES):
        xs = xf[cidx * TOK : (cidx + 1) * TOK]
        in_maps.append({"xT": pack_x_shard(xs, in_pairs), "w": w_flat,
                        "bias": bias_p})
    res = bass_utils.run_bass_kernel_spmd(
        nc, in_maps, core_ids=list(range(N_CORES)))
    fmap = out_feat_map(groups, set_G)
    outs = []
    for cidx in range(N_CORES):
        o = res.results[cidx]["outT"].reshape(OUT, TOK)
        unperm = np.empty_like(o)
        unperm[fmap] = o
        outs.append(unperm.T)
    full = np.concatenate(outs, axis=0)
    return np.ascontiguousarray(full.reshape(B, S, OUT).astype(np.float32))


def _run_dense(x, weight, bias, mask, nc):
    NM = OUT // P
    wm = (weight.astype(np.float32) * mask).astype(np.float16)
    w_packed = np.ascontiguousarray(
        wm.T.reshape(KO, P, NM, P).transpose(2, 1, 0, 3))
    bias_p = np.ascontiguousarray(bias.astype(np.float32).reshape(NM, P).T)
    B, S = x.shape[0], x.shape[1]
    xf = np.ascontiguousarray(x.reshape(B * S, IN))
    in_maps = []
    for cidx in range(N_CORES):
        xs = xf[cidx * TOK : (cidx + 1) * TOK].astype(np.float16)
        xp = np.ascontiguousarray(xs.T.reshape(KO, P, TOK).transpose(1, 0, 2))
        in_maps.append({"xT": xp, "w": w_packed, "bias": bias_p})
    res = bass_utils.run_bass_kernel_spmd(
        nc, in_maps, core_ids=list(range(N_CORES)))
    outs = []
    for cidx in range(N_CORES):
        o = res.results[cidx]["outT"].reshape(OUT, TOK)
        outs.append(o.T)
    full = np.concatenate(outs, axis=0)
    return np.ascontiguousarray(full.reshape(B, S, OUT).astype(np.float32))


def run_traced(inputs):
    """Dev-only traced timing run for test.py; the harness never calls this."""
    x = np.asarray(inputs["x"], dtype=np.float32)
    weight = np.asarray(inputs["weight"], dtype=np.float32)
    bias = np.asarray(inputs["bias"], dtype=np.float32)
    mask = np.asarray(inputs["mask"]).astype(bool)
    kernel(x, weight, bias, mask)  # ensure plan compiled+cached
    key = hash(mask.tobytes())
    kind, plan = _CACHE[key]
    if kind != "sparse":
        return None
    nc, in_pairs, groups, set_G, slot_lists, n_sr = plan
    w_flat = pack_weights(weight, mask, in_pairs, groups, set_G,
                          slot_lists, n_sr)
    bias_p = pack_bias(bias, groups, set_G)
    xf = np.ascontiguousarray(x.reshape(-1, IN))
    in_maps = []
    for cidx in range(N_CORES):
        xs = xf[cidx * TOK : (cidx + 1) * TOK]
        in_maps.append({"xT": pack_x_shard(xs, in_pairs), "w": w_flat,
                        "bias": bias_p})
    return bass_utils.run_bass_kernel_spmd(
        nc, in_maps, core_ids=list(range(N_CORES)), trace=True)


def kernel(x, weight, bias, mask):
    x = np.asarray(x, dtype=np.float32)
    weight = np.asarray(weight, dtype=np.float32)
    bias = np.asarray(bias, dtype=np.float32)
    mask = np.asarray(mask).astype(bool)
    assert x.shape == (4, 2048, IN) and weight.shape == (OUT, IN)

    key = hash(mask.tobytes())
    if key not in _CACHE:
        in_pairs, groups, sc64 = analyze_mask(mask)
        cells = int(sc64.sum())
        if cells <= SPARSE_MAX_CELLS:
            set_G, slot_lists, perm = build_schedule(sc64)
            in_pairs = in_pairs[perm]
            nc, n_sr = build_sparse(slot_lists)
            _CACHE[key] = ("sparse",
                           (nc, in_pairs, groups, set_G, slot_lists, n_sr))
        else:
            _CACHE[key] = ("dense", build_dense())
    kind, plan = _CACHE[key]
    if kind == "sparse":
        return _run_sparse(x, weight, bias, mask, plan)
    return _run_dense(x, weight, bias, mask, plan)

